# revision 1
# baseline (speedup 1.0000x reference)
"""StyleGAN2 modulated conv_transpose (stride=1, pad=1) for Trainium2.

Strategy (data-parallel over batch, 2 samples per core on 8 cores):
  conv_transpose2d(x, w_mod) with per-sample modulated+demodulated weights
  factors exactly as
      out_b[o] = (GAIN/d_b[o]) * conv2d(s_b (.) x_b, W*HE)[o] + GAIN*bias[o]
      d_b[o]   = sqrt(HE^2 * sum_i s_b[i]^2 * R[i,o] + eps),  R = sum_taps W^2
  so all samples share one weight tensor:
    - DVE: scale input channels by style (contiguous 32x32 images, no padding;
           conv boundary handled by shrunken matmul windows)
    - PE:  9 shifted-window matmuls x 4 k-tiles accumulate each (128 out x 512
           spatial) PSUM tile; demod norms via a tiny (N=2) PE matmul over R
    - ACT/DVE: copy-out fused with per-(sample,out) scale and bias
  Input DMAs are spread across the SP + ACT HWDGE queues and 4 SWDGE queues.
"""

from contextlib import ExitStack

import numpy as np

import concourse.bass as bass
from concourse import bacc
import concourse.mybir as mybir
import concourse.tile as tile
from concourse.bass_utils import run_bass_kernel_spmd

# matmul dtype mode: "f32" (exact, 4 cyc/row), "f32r" (fast fp32, 1 cyc/row),
# "bf16" (fast, ~2e-3 rel err)
MODE = "f32r"
TRACE = False
TRACE_KW = {}
LAST_RESULT = None

B, C, H, W, KK = 16, 512, 32, 32, 3
HW = H * W
NCORES, BPC = 8, B // 8
KT = C // 128  # k-tiles over in-channels
MT = C // 128  # m-tiles over out-channels
NT = 2         # spatial halves: N = 512 = 16 rows of 32
ROWS_N = H // NT
GAIN = 1.4142135623730951
HE = GAIN / float(C * KK * KK) ** 0.5
EPS = 1e-8

TAP_ORDER = [4, 0, 1, 2, 3, 5, 6, 7, 8]  # center tap first (full window)

F32 = mybir.dt.float32


def _build(mode):
    pad_dt = {"f32": F32, "f32r": mybir.dt.float32r, "bf16": mybir.dt.bfloat16}[mode]
    nc = bacc.Bacc("TRN2", target_bir_lowering=False, num_swdge_queues=4)
    x_d = nc.declare_dram_parameter("x", [BPC, C, HW], F32, isOutput=False)
    wt_d = nc.declare_dram_parameter("wt", [KK * KK, C, C], F32, isOutput=False)
    st_d = nc.declare_dram_parameter("style", [BPC, C], F32, isOutput=False)
    bi_d = nc.declare_dram_parameter("bias", [C], F32, isOutput=False)
    out_d = nc.declare_dram_parameter("out", [BPC, C, HW], F32, isOutput=True)

    with tile.TileContext(nc) as tc, ExitStack() as ctx:
        singles = ctx.enter_context(tc.tile_pool(name="singles", bufs=1))
        stage = ctx.enter_context(tc.tile_pool(name="stage", bufs=4))
        wstage = ctx.enter_context(tc.tile_pool(name="wstage", bufs=2))
        tmps = ctx.enter_context(tc.tile_pool(name="tmps", bufs=3))
        osbp = ctx.enter_context(tc.tile_pool(name="osbp", bufs=4))
        cpsum = ctx.enter_context(tc.tile_pool(name="cpsum", bufs=6, space="PSUM"))
        dpsum = ctx.enter_context(tc.tile_pool(name="dpsum", bufs=1, space="PSUM"))

        # ---- small constants: style, style^2, GAIN*bias ----
        s_t = singles.tile([128, KT, BPC], F32, tag="s_t")
        for b in range(BPC):
            nc.gpsimd.dma_start(
                out=s_t[:, :, b], in_=st_d[b].rearrange("(k p) -> p k", p=128)
            )
        s2_t = singles.tile([128, KT, BPC], F32, tag="s2_t")
        nc.vector.tensor_mul(s2_t, s_t, s_t)
        gb_t = singles.tile([128, MT], F32, tag="gb_t")
        nc.gpsimd.dma_start(out=gb_t, in_=bi_d[:].rearrange("(m p) -> p m", p=128))
        nc.vector.tensor_scalar_mul(gb_t, gb_t, float(GAIN))

        # ---- PE warmup: ~4us of dummy f32 matmuls on zeros releases the HAM
        # clock gate before real work arrives (PE runs 1.2 GHz cold, 2.4 warm)
        wz_t = singles.tile([128, 256], F32, tag="wz_t")
        nc.vector.memset(wz_t, 0.0)
        wps = dpsum.tile([128, ROWS_N, W], F32, tag="wps", name="wps")
        for _ in range(9):
            nc.tensor.matmul(
                wps.rearrange("p r w -> p (r w)")[:, :128],
                wz_t[:, :128],
                wz_t[:, 64:192],
                start=True,
                stop=True,
            )

        # ---- interleaved input/weight stream, in PE consumption order ----
        # x images: style-scaled (128, 32 rows, 34 cols), zero cols 0/33 (conv
        # col-padding; row padding via shrunken matmul windows).
        # weights: per-tap stage -> cast to matmul dtype + R = sum_taps W^2.
        zc_t = singles.tile([128, H, 2], F32, tag="zc_t")
        nc.vector.memset(zc_t, 0.0)
        engines = [nc.sync, nc.scalar, nc.gpsimd, nc.gpsimd]
        pads = {}
        w_mm = singles.tile([128, KK * KK, KT, C], pad_dt, tag="w_mm")
        R_t = singles.tile([128, KT, C], F32, tag="R_t")

        stream = [
            ("x", 0, 0), ("w", 0), ("x", 1, 0), ("w", 1),
            ("x", 2, 0), ("x", 3, 0), ("w", 2), ("w", 3),
            ("x", 0, 1), ("w", 4), ("x", 1, 1), ("w", 5),
            ("x", 2, 1), ("w", 6), ("x", 3, 1), ("w", 7), ("w", 8),
        ]

        for si, item in enumerate(stream):
            eng = engines[si % 4]
            if item[0] == "x":
                _, k, b = item
                xs = stage.tile([128, H, W], F32, tag="xs")
                eng.dma_start(
                    out=xs,
                    in_=x_d[b].rearrange("(k p) (h w) -> k p h w", p=128, h=H)[k],
                )
                pt = singles.tile([128, H, W + 2], pad_dt, tag=f"pad_{b}_{k}")
                nc.vector.tensor_scalar_mul(
                    pt[:, :, 1 : W + 1], xs, s_t[:, k, b : b + 1]
                )
                # zero columns 0 and 33 in one strided copy
                border = bass.AP(
                    tensor=pt.tensor,
                    offset=pt.offset,
                    ap=[pt.ap[0], [W + 2, H], [W + 1, 2]],
                )
                nc.vector.tensor_copy(out=border, in_=zc_t)
                pads[b, k] = pt
            else:
                _, ti = item
                t = TAP_ORDER[ti]
                if mode == "f32":
                    ws = w_mm[:, t]
                else:
                    ws = wstage.tile([128, KT, C], F32, tag="ws")
                eng.dma_start(
                    out=ws, in_=wt_d[t].rearrange("(k p) o -> p k o", p=128)
                )
                if mode != "f32":
                    nc.vector.tensor_copy(out=w_mm[:, t], in_=ws)
                for k in range(KT):
                    if ti == 0:
                        nc.scalar.square(R_t[:, k], ws[:, k])
                    else:
                        sq = tmps.tile([128, C], F32, tag="sq")
                        nc.scalar.square(sq, ws[:, k])
                        nc.vector.tensor_add(R_t[:, k], R_t[:, k], sq)

        dinv = singles.tile([128, MT, BPC], F32, tag="dinv")

        # ---- conv: 3 phases of up to 6 (b, m) tile-groups x 2 n-tiles,
        # using 6 PSUM banks (+1 warmup, +1 demod-norm bank). Phase 0 is
        # sample 0 only and its (tap,k) pairs are ordered by estimated DMA
        # arrival so the PE never out-runs the input stream.
        out_engines = [nc.sync, nc.scalar]
        oi = 0
        # estimated delivery (us) per stream position at ~0.32 B/ns
        xd = {0: 1.6, 1: 6.3, 2: 10.9, 3: 12.5}
        wd = {0: 4.7, 1: 9.4, 2: 15.6, 3: 18.8, 4: 23.4, 5: 28.1, 6: 32.8, 7: 37.5, 8: 40.6}
        pairs_sorted = sorted(
            ((ti, k) for ti in range(KK * KK) for k in range(KT)),
            key=lambda p: (max(wd[p[0]], xd[p[1]]), p[0], p[1]),
        )
        pairs_nat = [(ti, k) for ti in range(KK * KK) for k in range(KT)]
        PHASES = [
            (pairs_sorted, [(0, 0), (0, 1), (0, 2)]),
            (pairs_nat, [(0, 3), (1, 0), (1, 1)]),
            (pairs_nat, [(1, 2), (1, 3)]),
        ]
        for pi, (pairs, groups) in enumerate(PHASES):
            cps = {}
            for g in groups:
                for n in range(NT):
                    cp = cpsum.tile([128, ROWS_N, W], F32, tag="cps")
                    cps[g, n] = cp
            started = set()
            npairs = len(pairs)
            for pidx, (ti, k) in enumerate(pairs):
                t = TAP_ORDER[ti]
                a, bw = divmod(t, 3)
                h_lo_g, h_hi_g = max(0, a - 1), min(H, H - 1 + a)
                last = pidx == npairs - 1
                for g in groups:
                    b, m = g
                    pt = pads[b, k]
                    lhsT = w_mm[:, t, k, m * 128 : (m + 1) * 128]
                    for n in range(NT):
                        h_lo = max(n * ROWS_N, h_lo_g)
                        h_hi = min((n + 1) * ROWS_N, h_hi_g)
                        out_ap = cps[g, n][
                            :, h_lo - n * ROWS_N : h_hi - n * ROWS_N, :
                        ]
                        rhs = pt[
                            :,
                            h_lo + 1 - a : h_hi + 1 - a,
                            2 - bw : 2 - bw + W,
                        ]
                        first = (g, n) not in started
                        if first:
                            assert t == 4, "start matmul must cover full tile"
                            started.add((g, n))
                        nc.tensor.matmul(
                            out_ap,
                            lhsT,
                            rhs,
                            start=first,
                            stop=last,
                        )
            if pi == 0:
                # demod norms: d2[o, bb] = sum_i s2[i,bb] * R[i,o]
                d2p = dpsum.tile([128, MT, BPC], F32, tag="d2p")
                for m2 in range(MT):
                    for k in range(KT):
                        nc.tensor.matmul(
                            d2p[:, m2],
                            R_t[:, k, m2 * 128 : (m2 + 1) * 128],
                            s2_t[:, k],
                            start=(k == 0),
                            stop=(k == KT - 1),
                        )
                # dinv = GAIN*HE/sqrt(HE^2*d2+EPS) = 1/sqrt(d2/G^2 + EPS/(HE*G)^2)
                dsq = singles.tile([128, MT, BPC], F32, tag="dsq")
                eps_t = singles.tile([128, 1], F32, tag="eps_t")
                nc.vector.memset(eps_t, float(EPS / (HE * HE * GAIN * GAIN)))
                nc.scalar.activation(
                    dsq,
                    d2p,
                    mybir.ActivationFunctionType.Sqrt,
                    bias=eps_t,
                    scale=float(1.0 / (GAIN * GAIN)),
                )
                nc.vector.reciprocal(dinv, dsq)
            for g in groups:
                b, m = g
                for n in range(NT):
                    osb = osbp.tile([128, ROWS_N * W], F32, tag="osb")
                    cp_flat = cps[g, n].rearrange("p r w -> p (r w)")
                    if (m + n) % 2 == 0:
                        nc.scalar.activation(
                            osb,
                            cp_flat,
                            mybir.ActivationFunctionType.Identity,
                            bias=gb_t[:, m : m + 1],
                            scale=dinv[:, m, b : b + 1],
                        )
                    else:
                        nc.vector.tensor_scalar(
                            osb,
                            cp_flat,
                            dinv[:, m, b : b + 1],
                            gb_t[:, m : m + 1],
                            op0=mybir.AluOpType.mult,
                            op1=mybir.AluOpType.add,
                        )
                    out_engines[oi % 2].dma_start(
                        out=out_d[b].rearrange("(mm p) s -> mm p s", p=128)[m][
                            :, n * ROWS_N * W : (n + 1) * ROWS_N * W
                        ],
                        in_=osb,
                    )
                    oi += 1
    nc.finalize()
    return nc


def kernel(inp, style, weight, bias):
    global LAST_RESULT
    inp = np.ascontiguousarray(np.asarray(inp, np.float32)).reshape(B, C, HW)
    w_t = np.ascontiguousarray(
        np.asarray(weight, np.float32).transpose(2, 3, 0, 1)
    ).reshape(KK * KK, C, C)
    style = np.ascontiguousarray(np.asarray(style, np.float32))
    bias = np.ascontiguousarray(np.asarray(bias, np.float32))

    nc = _build(MODE)
    in_maps = []
    for c in range(NCORES):
        sl = slice(c * BPC, (c + 1) * BPC)
        in_maps.append(
            {"x": inp[sl], "wt": w_t, "style": style[sl], "bias": bias}
        )
    res = run_bass_kernel_spmd(
        nc, in_maps, list(range(NCORES)), trace=TRACE, **TRACE_KW
    )
    LAST_RESULT = res
    out = np.concatenate([res.results[c]["out"] for c in range(NCORES)], axis=0)
    return out.reshape(B, C, H, W)



# revision 8
# speedup vs baseline: 1.1190x; 1.1190x over previous
"""StyleGAN2 modulated conv_transpose (stride=1, pad=1) for Trainium2.

Strategy (data-parallel over batch, 2 samples per core on 8 cores):
  conv_transpose2d(x, w_mod) with per-sample modulated+demodulated weights
  factors exactly as
      out_b[o] = (GAIN/d_b[o]) * conv2d(s_b (.) x_b, W*HE)[o] + GAIN*bias[o]
      d_b[o]   = sqrt(HE^2 * sum_i s_b[i]^2 * R[i,o] + eps),  R = sum_taps W^2
  so all samples share one weight tensor:
    - DVE: scale input channels by style (contiguous 32x32 images, no padding;
           conv boundary handled by shrunken matmul windows)
    - PE:  9 shifted-window matmuls x 4 k-tiles accumulate each (128 out x 512
           spatial) PSUM tile; demod norms via a tiny (N=2) PE matmul over R
    - ACT/DVE: copy-out fused with per-(sample,out) scale and bias
  Input DMAs are spread across the SP + ACT HWDGE queues and 4 SWDGE queues.
"""

from contextlib import ExitStack

import numpy as np

import concourse.bass as bass
from concourse import bacc
import concourse.mybir as mybir
import concourse.tile as tile
from concourse.bass_utils import run_bass_kernel_spmd

# matmul dtype mode: "f32" (exact, 4 cyc/row), "f32r" (fast fp32, 1 cyc/row),
# "bf16" (fast, ~2e-3 rel err, casts on device), "bf16h" (fast, host-casts
# x/w to bf16: halves input DMA and speeds up LDWEIGHTS)
MODE = "bf16h"
TRACE = False
TRACE_KW = {}
LAST_RESULT = None

B, C, H, W, KK = 16, 512, 32, 32, 3
HW = H * W
NCORES, BPC = 8, B // 8
KT = C // 128  # k-tiles over in-channels
MT = C // 128  # m-tiles over out-channels
NT = 2         # spatial halves: N = 512 = 16 rows of 32
ROWS_N = H // NT
GAIN = 1.4142135623730951
HE = GAIN / float(C * KK * KK) ** 0.5
EPS = 1e-8

TAP_ORDER = [4, 0, 1, 2, 3, 5, 6, 7, 8]  # center tap first (full window)

F32 = mybir.dt.float32


def _build(mode):
    pad_dt = {
        "f32": F32,
        "f32r": mybir.dt.float32r,
        "bf16": mybir.dt.bfloat16,
        "bf16h": mybir.dt.bfloat16,
    }[mode]
    in_dt = mybir.dt.bfloat16 if mode == "bf16h" else F32
    nc = bacc.Bacc("TRN2", target_bir_lowering=False, num_swdge_queues=4)
    x_d = nc.declare_dram_parameter("x", [BPC, C, HW], in_dt, isOutput=False)
    wt_d = nc.declare_dram_parameter("wt", [KK * KK, C, C], in_dt, isOutput=False)
    st_d = nc.declare_dram_parameter("style", [BPC, C], F32, isOutput=False)
    bi_d = nc.declare_dram_parameter("bias", [C], F32, isOutput=False)
    out_d = nc.declare_dram_parameter("out", [BPC, C, HW], F32, isOutput=True)

    with tile.TileContext(nc) as tc, ExitStack() as ctx:
        singles = ctx.enter_context(tc.tile_pool(name="singles", bufs=1))
        stage = ctx.enter_context(tc.tile_pool(name="stage", bufs=4))
        wstage = ctx.enter_context(tc.tile_pool(name="wstage", bufs=2))
        tmps = ctx.enter_context(tc.tile_pool(name="tmps", bufs=3))
        osbp = ctx.enter_context(tc.tile_pool(name="osbp", bufs=4))
        cpsum = ctx.enter_context(tc.tile_pool(name="cpsum", bufs=6, space="PSUM"))
        dpsum = ctx.enter_context(tc.tile_pool(name="dpsum", bufs=1, space="PSUM"))

        # ---- small constants: style, style^2, GAIN*bias ----
        s_t = singles.tile([128, KT, BPC], F32, tag="s_t")
        for b in range(BPC):
            nc.gpsimd.dma_start(
                out=s_t[:, :, b], in_=st_d[b].rearrange("(k p) -> p k", p=128)
            )
        s2_t = singles.tile([128, KT, BPC], F32, tag="s2_t")
        nc.vector.tensor_mul(s2_t, s_t, s_t)
        gb_t = singles.tile([128, MT], F32, tag="gb_t")
        nc.gpsimd.dma_start(out=gb_t, in_=bi_d[:].rearrange("(m p) -> p m", p=128))
        nc.vector.tensor_scalar_mul(gb_t, gb_t, float(GAIN))

        # ---- PE warmup: ~4us of dummy f32 matmuls on zeros releases the HAM
        # clock gate before real work arrives (PE runs 1.2 GHz cold, 2.4 warm)
        wz_t = singles.tile([128, 256], F32, tag="wz_t")
        nc.vector.memset(wz_t, 0.0)
        wps = dpsum.tile([128, ROWS_N, W], F32, tag="wps", name="wps")
        for _ in range(9):
            nc.tensor.matmul(
                wps.rearrange("p r w -> p (r w)")[:, :128],
                wz_t[:, :128],
                wz_t[:, 64:192],
                start=True,
                stop=True,
            )

        # ---- interleaved input/weight stream, in PE consumption order ----
        # x images: style-scaled (128, 32 rows, 34 cols), zero cols 0/33 (conv
        # col-padding; row padding via shrunken matmul windows).
        # weights: per-tap stage -> cast to matmul dtype + R = sum_taps W^2.
        zc_t = singles.tile([128, H, 2], pad_dt, tag="zc_t")
        nc.vector.memset(zc_t, 0.0)
        engines = [nc.sync, nc.scalar, nc.gpsimd, nc.gpsimd]
        pads = {}
        w_mm = singles.tile([128, KK * KK, KT, C], pad_dt, tag="w_mm")
        R_t = singles.tile([128, KT, C], F32, tag="R_t")

        stream = [
            ("x", 0, 0), ("w", 0), ("x", 1, 0), ("w", 1),
            ("x", 2, 0), ("x", 3, 0), ("w", 2), ("w", 3),
            ("x", 0, 1), ("w", 4), ("x", 1, 1), ("w", 5),
            ("x", 2, 1), ("w", 6), ("x", 3, 1), ("w", 7), ("w", 8),
        ]

        for si, item in enumerate(stream):
            eng = engines[si % 4]
            if item[0] == "x":
                _, k, b = item
                xs = stage.tile([128, H, W], in_dt, tag="xs")
                eng.dma_start(
                    out=xs,
                    in_=x_d[b].rearrange("(k p) (h w) -> k p h w", p=128, h=H)[k],
                )
                pt = singles.tile([128, H, W + 2], pad_dt, tag=f"pad_{b}_{k}")
                nc.vector.tensor_scalar_mul(
                    pt[:, :, 1 : W + 1], xs, s_t[:, k, b : b + 1]
                )
                # zero columns 0 and 33 in one strided copy
                border = bass.AP(
                    tensor=pt.tensor,
                    offset=pt.offset,
                    ap=[pt.ap[0], [W + 2, H], [W + 1, 2]],
                )
                nc.vector.tensor_copy(out=border, in_=zc_t)
                pads[b, k] = pt
            else:
                _, ti = item
                t = TAP_ORDER[ti]
                if mode in ("f32", "bf16h"):
                    ws = w_mm[:, t]
                else:
                    ws = wstage.tile([128, KT, C], F32, tag="ws")
                eng.dma_start(
                    out=ws, in_=wt_d[t].rearrange("(k p) o -> p k o", p=128)
                )
                if mode not in ("f32", "bf16h"):
                    nc.vector.tensor_copy(out=w_mm[:, t], in_=ws)
                for k in range(KT):
                    if ti == 0:
                        nc.scalar.square(R_t[:, k], ws[:, k])
                    else:
                        sq = tmps.tile([128, C], F32, tag="sq")
                        nc.scalar.square(sq, ws[:, k])
                        nc.vector.tensor_add(R_t[:, k], R_t[:, k], sq)

        dinv = singles.tile([128, MT, BPC], F32, tag="dinv")

        # ---- conv: 3 phases of up to 6 (b, m) tile-groups x 2 n-tiles,
        # using 6 PSUM banks (+1 warmup, +1 demod-norm bank). Phase 0 is
        # sample 0 only and its (tap,k) pairs are ordered by estimated DMA
        # arrival so the PE never out-runs the input stream.
        out_engines = [nc.sync, nc.scalar]
        oi = 0
        # estimated delivery (us) per stream position at ~0.32 B/ns
        xd = {0: 1.6, 1: 6.3, 2: 10.9, 3: 12.5}
        wd = {0: 4.7, 1: 9.4, 2: 15.6, 3: 18.8, 4: 23.4, 5: 28.1, 6: 32.8, 7: 37.5, 8: 40.6}
        if mode == "bf16h":  # 2-byte stream arrives twice as fast
            xd = {k: v / 2 for k, v in xd.items()}
            wd = {k: v / 2 for k, v in wd.items()}
        pairs_sorted = sorted(
            ((ti, k) for ti in range(KK * KK) for k in range(KT)),
            key=lambda p: (max(wd[p[0]], xd[p[1]]), p[0], p[1]),
        )
        pairs_nat = [(ti, k) for ti in range(KK * KK) for k in range(KT)]
        PHASES = [
            (pairs_sorted, [(0, 0), (0, 1), (0, 2)]),
            (pairs_nat, [(0, 3), (1, 0), (1, 1)]),
            (pairs_nat, [(1, 2), (1, 3)]),
        ]
        for pi, (pairs, groups) in enumerate(PHASES):
            cps = {}
            for g in groups:
                for n in range(NT):
                    cp = cpsum.tile([128, ROWS_N, W], F32, tag="cps")
                    cps[g, n] = cp
            started = set()
            npairs = len(pairs)
            for pidx, (ti, k) in enumerate(pairs):
                t = TAP_ORDER[ti]
                a, bw = divmod(t, 3)
                h_lo_g, h_hi_g = max(0, a - 1), min(H, H - 1 + a)
                last = pidx == npairs - 1
                for g in groups:
                    b, m = g
                    pt = pads[b, k]
                    lhsT = w_mm[:, t, k, m * 128 : (m + 1) * 128]
                    for n in range(NT):
                        h_lo = max(n * ROWS_N, h_lo_g)
                        h_hi = min((n + 1) * ROWS_N, h_hi_g)
                        out_ap = cps[g, n][
                            :, h_lo - n * ROWS_N : h_hi - n * ROWS_N, :
                        ]
                        rhs = pt[
                            :,
                            h_lo + 1 - a : h_hi + 1 - a,
                            2 - bw : 2 - bw + W,
                        ]
                        first = (g, n) not in started
                        if first:
                            assert t == 4, "start matmul must cover full tile"
                            started.add((g, n))
                        nc.tensor.matmul(
                            out_ap,
                            lhsT,
                            rhs,
                            start=first,
                            stop=last,
                        )
            if pi == 0:
                # demod norms: d2[o, bb] = sum_i s2[i,bb] * R[i,o]
                d2p = dpsum.tile([128, MT, BPC], F32, tag="d2p")
                for m2 in range(MT):
                    for k in range(KT):
                        nc.tensor.matmul(
                            d2p[:, m2],
                            R_t[:, k, m2 * 128 : (m2 + 1) * 128],
                            s2_t[:, k],
                            start=(k == 0),
                            stop=(k == KT - 1),
                        )
                # dinv = GAIN*HE/sqrt(HE^2*d2+EPS) = 1/sqrt(d2/G^2 + EPS/(HE*G)^2)
                dsq = singles.tile([128, MT, BPC], F32, tag="dsq")
                eps_t = singles.tile([128, 1], F32, tag="eps_t")
                nc.vector.memset(eps_t, float(EPS / (HE * HE * GAIN * GAIN)))
                nc.scalar.activation(
                    dsq,
                    d2p,
                    mybir.ActivationFunctionType.Sqrt,
                    bias=eps_t,
                    scale=float(1.0 / (GAIN * GAIN)),
                )
                nc.vector.reciprocal(dinv, dsq)
            for g in groups:
                b, m = g
                for n in range(NT):
                    osb = osbp.tile([128, ROWS_N * W], F32, tag="osb")
                    cp_flat = cps[g, n].rearrange("p r w -> p (r w)")
                    if (m + n) % 2 == 0:
                        nc.scalar.activation(
                            osb,
                            cp_flat,
                            mybir.ActivationFunctionType.Identity,
                            bias=gb_t[:, m : m + 1],
                            scale=dinv[:, m, b : b + 1],
                        )
                    else:
                        nc.vector.tensor_scalar(
                            osb,
                            cp_flat,
                            dinv[:, m, b : b + 1],
                            gb_t[:, m : m + 1],
                            op0=mybir.AluOpType.mult,
                            op1=mybir.AluOpType.add,
                        )
                    out_engines[oi % 2].dma_start(
                        out=out_d[b].rearrange("(mm p) s -> mm p s", p=128)[m][
                            :, n * ROWS_N * W : (n + 1) * ROWS_N * W
                        ],
                        in_=osb,
                    )
                    oi += 1
    nc.finalize()
    return nc


def kernel(inp, style, weight, bias):
    global LAST_RESULT
    inp = np.ascontiguousarray(np.asarray(inp, np.float32)).reshape(B, C, HW)
    w_t = np.ascontiguousarray(
        np.asarray(weight, np.float32).transpose(2, 3, 0, 1)
    ).reshape(KK * KK, C, C)
    style = np.ascontiguousarray(np.asarray(style, np.float32))
    bias = np.ascontiguousarray(np.asarray(bias, np.float32))
    if MODE == "bf16h":
        import ml_dtypes

        inp = inp.astype(ml_dtypes.bfloat16)
        w_t = w_t.astype(ml_dtypes.bfloat16)

    nc = _build(MODE)
    in_maps = []
    for c in range(NCORES):
        sl = slice(c * BPC, (c + 1) * BPC)
        in_maps.append(
            {"x": inp[sl], "wt": w_t, "style": style[sl], "bias": bias}
        )
    res = run_bass_kernel_spmd(
        nc, in_maps, list(range(NCORES)), trace=TRACE, **TRACE_KW
    )
    LAST_RESULT = res
    out = np.concatenate([res.results[c]["out"] for c in range(NCORES)], axis=0)
    return out.reshape(B, C, H, W)

